# revision 10
# baseline (speedup 1.0000x reference)
"""3-layer GAT on 8 Trainium2 NeuronCores (Bass/Tile), v2.

Sharding: nodes by contiguous range (6250/core); edges by dst range, sorted
by (dst, src). Per layer: dense phase computes [feat|el|er] = h @ [W|W.al|W.ar]
for local nodes into an fp16 row table -> AllGather the table -> edge phase
gathers table[src] rows (dma_gather on 4 SWDGE queues so descriptor generation
runs on all four Q7 core pairs), builds one-hot(dst) matrices on DVE in fp16,
broadcasts er via DMA-transposed one-hots + tiny matmuls, computes
exp(leaky_relu(el+er)) on ACT, and aggregates (weighted feature sum + softmax
denominator) in one fp16 matmul chain per 128-dst-node window into PSUM.
Epilogue normalizes, adds residual, applies ELU (or head-mean for the output
layer) and writes the next layer's input directly into SBUF (h never
round-trips DRAM). Dense phase of layer l+1 is interleaved with edge phase of
layer l per window.

All feature storage/movement is fp16 (table rows 768B / 512B vs f32 1280B),
halving gather+AllGather traffic; accumulation stays f32 in PSUM.

Softmax is computed without the segment-max subtraction: attention logits are
O(1) here so exp() cannot overflow, and the result is mathematically
identical.
"""
import sys

sys.path.insert(0, "/opt/trn_rl_repo")

import numpy as np

# ---- problem constants (nn_GAT_3951369912452) ----
N = 50000
E = 800000
IN = 256
HID = 64
H = 4
C = 40
SLOPE = 0.2
NCORES = 8
NLOC = N // NCORES          # 6250
P = 128
W = (NLOC + P - 1) // P     # 49 windows/core
SPLIT = 32768               # int16 gather index limit

F0 = H * HID                # 256 feat width, layers 0/1
F2 = H * C                  # 160 feat width, layer 2
ROW0 = 384                  # fp16 row elems, layers 0/1 (768B)
ROW2 = 256                  # fp16 row elems, layer 2 (512B)
EL0 = 256                   # el col offset in rows / psd, layers 0/1
EL2 = 160                   # layer 2
CD = (F0 + 8, F0 + 8, F2 + 8 + F2)   # dense matmul out cols per layer
ROWS = (ROW0, ROW0, ROW2)
ELS = (EL0, EL0, EL2)
FS = (F0, F0, F2)
RHSS = (F0 + 4, F0 + 4, F2 + 4)

_CACHE = {}


# ======================= host preprocessing =======================

def _fold_w(Wm, al, ar):
    Hh, D = al.shape
    Wal = np.stack([Wm[:, h * D:(h + 1) * D] @ al[h] for h in range(Hh)], axis=1)
    War = np.stack([Wm[:, h * D:(h + 1) * D] @ ar[h] for h in range(Hh)], axis=1)
    return Wal.astype(np.float32), War.astype(np.float32)


def _wrap16(block):
    """int16 idx list (cap,) -> dma_gather wrapped layout (128, cap//16)."""
    cap = block.shape[0]
    wb = block.reshape(cap // 16, 16).T
    return np.tile(wb, (8, 1)).astype(np.int16)


def _preprocess(inputs):
    x = np.asarray(inputs["x"], np.float32)
    src = np.asarray(inputs["src"], np.int64)
    dst = np.asarray(inputs["dst"], np.int64)

    Wcat = []
    for l, (Wm, al, ar) in enumerate(
        [(inputs["W0"], inputs["al0"], inputs["ar0"]),
         (inputs["W1"], inputs["al1"], inputs["ar1"]),
         (inputs["W2"], inputs["al2"], inputs["ar2"])]
    ):
        Wm = np.asarray(Wm, np.float32)
        Wal, War = _fold_w(Wm, np.asarray(al, np.float32), np.asarray(ar, np.float32))
        parts = [Wm, Wal, War]
        if l == 2:
            # residual projection, pre-scaled by the head-mean 1/H factor
            parts.append(np.asarray(inputs["Wres2"], np.float32) / H)
        Wcat.append(np.ascontiguousarray(
            np.concatenate(parts, axis=1)).astype(np.float16))

    order = np.argsort(dst, kind="stable")
    ds = dst[order]
    ss = src[order]

    # per (core, window) edge lists, split at SPLIT, sorted by src
    per_core = []
    KA = KB = 1
    for r in range(NCORES):
        lo = r * NLOC
        m = (ds >= lo) & (ds < lo + NLOC)
        ld = ds[m] - lo
        ls = ss[m]
        wins = []
        for w in range(W):
            wm = (ld >= w * P) & (ld < (w + 1) * P)
            dw = ld[wm] - w * P
            sw = ls[wm]
            so = np.argsort(sw, kind="stable")
            sw, dw = sw[so], dw[so]
            a = sw < SPLIT
            sa, da = sw[a], dw[a]
            sb, db = sw[~a] - SPLIT, dw[~a]
            wins.append((sa, da, sb, db))
            KA = max(KA, -(-max(len(sa), 1) // P))
            KB = max(KB, -(-max(len(sb), 1) // P))
        per_core.append(wins)
    KT = KA + KB

    in_maps = []
    for r in range(NCORES):
        idxA = np.full((W, KA * P), -1, np.int16)
        idxB = np.full((W, KB * P), -1, np.int16)
        drel = np.full((W, KT * P), -1.0, np.float32)
        meta = np.zeros((2 * W,), np.int32)
        for w, (sa, da, sb, db) in enumerate(per_core[r]):
            na, nb = len(sa), len(sb)
            idxA[w, :na] = sa.astype(np.int16)
            idxB[w, :nb] = sb.astype(np.int16)
            if na == 0:
                idxA[w, 0] = 0
            if nb == 0:
                idxB[w, 0] = 0
            meta[w] = max(na, 1)
            meta[W + w] = max(nb, 1)
            drel[w, :na] = da.astype(np.float32)
            drel[w, KA * P:KA * P + nb] = db.astype(np.float32)

        in_maps.append({
            "x": np.ascontiguousarray(x[r * NLOC:(r + 1) * NLOC]).astype(np.float16),
            "Wcat0": Wcat[0], "Wcat1": Wcat[1], "Wcat2": Wcat[2],
            "idxA": np.hstack([_wrap16(idxA[w]) for w in range(W)]),
            "idxB": np.hstack([_wrap16(idxB[w]) for w in range(W)]),
            "drel": np.hstack([drel[w].reshape(KT, P).T for w in range(W)]
                              ).astype(np.float16),
            "meta": meta.reshape(1, 2 * W),
        })

    meta_prog = {"KA": KA, "KB": KB}
    return in_maps, meta_prog


# ======================= device program =======================

def _build(meta_prog, dbg=False):
    import concourse.bass as bass
    import concourse.bacc as bacc
    import concourse.mybir as mybir
    import concourse.tile as tile

    KA, KB = meta_prog["KA"], meta_prog["KB"]
    KT = KA + KB
    f32 = mybir.dt.float32
    f16 = mybir.dt.float16
    i16 = mybir.dt.int16
    i32 = mybir.dt.int32
    AF = mybir.ActivationFunctionType
    OP = mybir.AluOpType

    nc = bacc.Bacc("TRN2", target_bir_lowering=False, debug=False,
                   num_devices=NCORES, num_swdge_queues=4)

    # ---- I/O ----
    x_d = nc.dram_tensor("x", [NLOC, F0], f16, kind="ExternalInput")
    Wc_d = [nc.dram_tensor(f"Wcat{l}", [IN, CD[l]], f16, kind="ExternalInput")
            for l in range(3)]
    idxA_d = nc.dram_tensor("idxA", [P, W * KA * 8], i16, kind="ExternalInput")
    idxB_d = nc.dram_tensor("idxB", [P, W * KB * 8], i16, kind="ExternalInput")
    drel_d = nc.dram_tensor("drel", [P, W * KT], f16, kind="ExternalInput")
    meta_d = nc.dram_tensor("meta", [1, 2 * W], i32, kind="ExternalInput")
    out_d = nc.dram_tensor("out", [NLOC, C], f32, kind="ExternalOutput")

    # ---- internal DRAM ----
    tinf = [nc.dram_tensor(f"tin{l}", [(NLOC + 2) * ROWS[l]], f16)
            for l in range(3)]
    tab = [nc.dram_tensor(f"tab{l}", [N, ROWS[l]], f16, addr_space="Shared")
           for l in range(3)]
    tin_rows = [tinf[l][0:NLOC * ROWS[l]].rearrange("(n c) -> n c", c=ROWS[l])
                for l in range(3)]

    with tile.TileContext(nc) as tc:
        with (
            tc.tile_pool(name="const", bufs=1) as cp,
            tc.tile_pool(name="work", bufs=2) as wp,
            tc.tile_pool(name="psum", bufs=2, space="PSUM") as pp,
        ):
            # ---- persistent tiles ----
            iota_f = cp.tile([P, P], f32)
            nc.gpsimd.iota(iota_f[:], pattern=[[1, P]], base=0,
                           channel_multiplier=0,
                           allow_small_or_imprecise_dtypes=True)
            iota16 = cp.tile([P, P], f16)
            nc.vector.tensor_copy(out=iota16[:], in_=iota_f[:])

            idxA_t = cp.tile([P, W * KA * 8], i16)
            nc.sync.dma_start(idxA_t[:], idxA_d[:, :])
            idxB_t = cp.tile([P, W * KB * 8], i16)
            nc.sync.dma_start(idxB_t[:], idxB_d[:, :])
            drel_t = cp.tile([P, W * KT], f16)
            nc.sync.dma_start(drel_t[:], drel_d[:, :])
            meta_t = cp.tile([1, 2 * W], i32)
            nc.sync.dma_start(meta_t[:], meta_d[:, :])

            Wc_t = []
            for l in range(3):
                chunks = []
                for k in range(2):
                    t = cp.tile([P, CD[l]], f16, tag=f"wc{l}{k}")
                    nc.sync.dma_start(t[:], Wc_d[l][k * P:(k + 1) * P, :])
                    chunks.append(t)
                Wc_t.append(chunks)

            # h buffer, SBUF-resident across all layers
            h_sb = cp.tile([P, W * F0], f16)
            # zero the last window once so its pad lanes (beyond the x/h DMA)
            # stay finite (er columns feed matmuls)
            nc.vector.memset(h_sb[:, (W - 1) * F0:W * F0].bitcast(f32), 0.0)

            er_all = [cp.tile([P, W * 4], f32, name=f"er_all{i}", tag=f"er{i}")
                      for i in range(2)]
            res2_sb = cp.tile([P, W * F2], f16)

            # gather destinations: zeroed once; stale lanes are masked by the
            # one-hot (0 rows) so they only need to stay finite
            GA = []
            GB = []
            for i in range(2):
                ga = cp.tile([P, KA * ROW0], f16, tag=f"GA{i}")
                nc.vector.memset(ga[:].bitcast(f32), 0.0)
                GA.append(ga)
                gb = cp.tile([P, KB * ROW0], f16, tag=f"GB{i}")
                nc.vector.memset(gb[:].bitcast(f32), 0.0)
                GB.append(gb)

            regs = [nc.gpsimd.alloc_register(f"gr{i}") for i in range(8)]

            def _tap(name, ap, shape):
                d = nc.dram_tensor(name, shape, ap.dtype, kind="ExternalOutput")
                nc.sync.dma_start(d[tuple(slice(0, s) for s in shape)], ap)

            def dense_win(l, w, load_x=False):
                nw = min(P, NLOC - w * P)
                if load_x:
                    nc.sync.dma_start(h_sb[0:nw, w * F0:(w + 1) * F0],
                                      x_d[w * P:w * P + nw, :])
                psd = pp.tile([P, CD[l]], f32, tag="psd")
                for k in range(2):
                    hT = wp.tile([P, P], f16, tag="hT")
                    nc.scalar.dma_start_transpose(
                        hT[:], h_sb[:, w * F0 + k * P:w * F0 + (k + 1) * P])
                    nc.tensor.matmul(out=psd[:], lhsT=hT[:], rhs=Wc_t[l][k][:],
                                     start=(k == 0), stop=(k == 1))
                EL = ELS[l]
                nc.vector.tensor_copy(out=er_all[l % 2][:, w * 4:(w + 1) * 4],
                                      in_=psd[:, EL + 4:EL + 8])
                do = wp.tile([P, ROW0], f16, tag="do")
                nc.vector.tensor_copy(out=do[:, 0:EL + 4], in_=psd[:, 0:EL + 4])
                nc.sync.dma_start(tin_rows[l][w * P:w * P + nw, :],
                                  do[0:nw, 0:ROWS[l]])
                if l == 2:
                    nc.vector.tensor_copy(
                        out=res2_sb[:, w * F2:(w + 1) * F2],
                        in_=psd[:, EL + 8:EL + 8 + F2])

            def edge_win(l, w, dbg=False):
                ROW, EL, F, RHS = ROWS[l], ELS[l], FS[l], RHSS[l]
                D = F // H
                par = w % 2
                nw = min(P, NLOC - w * P)
                ga = GA[par][:, 0:KA * ROW].rearrange("p (t c) -> p t c", c=ROW)
                gb = GB[par][:, 0:KB * ROW].rearrange("p (t c) -> p t c", c=ROW)
                rA = regs[(2 * w) % 8]
                rB = regs[(2 * w + 1) % 8]
                nc.gpsimd.reg_load(rA, meta_t[0:1, w:w + 1])
                nc.gpsimd.dma_gather(
                    out_ap=ga, in_ap=tab[l][0:SPLIT, :],
                    idxs_ap=idxA_t[:, w * KA * 8:(w + 1) * KA * 8],
                    num_idxs=KA * P, num_idxs_reg=rA, elem_size=ROW,
                    single_packet=False, queue_num=2 * par)
                nc.gpsimd.reg_load(rB, meta_t[0:1, W + w:W + w + 1])
                nc.gpsimd.dma_gather(
                    out_ap=gb, in_ap=tab[l][SPLIT:N, :],
                    idxs_ap=idxB_t[:, w * KB * 8:(w + 1) * KB * 8],
                    num_idxs=KB * P, num_idxs_reg=rB, elem_size=ROW,
                    single_packet=False, queue_num=2 * par + 1)

                oh = wp.tile([P, KT, P], f16, tag="oh")
                nc.vector.tensor_tensor(
                    out=oh[:, :, :],
                    in0=iota16[:, None, :].to_broadcast([P, KT, P]),
                    in1=drel_t[:, w * KT:(w + 1) * KT, None].to_broadcast(
                        [P, KT, P]),
                    op=OP.is_equal)
                ohT = wp.tile([P, KT, P], f16, tag="ohT")
                for t in range(KT):
                    nc.sync.dma_start_transpose(ohT[:, t, :], oh[:, t, :])

                er_w = wp.tile([P, 4], f16, tag="er_w")
                nc.vector.tensor_copy(out=er_w[:],
                                      in_=er_all[l % 2][:, w * 4:(w + 1) * 4])
                er_ps = pp.tile([P, KT, 4], f32, tag="er_ps")
                for t in range(KT):
                    nc.tensor.matmul(out=er_ps[:, t, :], lhsT=ohT[:, t, :],
                                     rhs=er_w[:], start=True, stop=True)

                ext = wp.tile([P, KT, 4], f32, tag="ext")
                nc.vector.tensor_copy(out=ext[:, 0:KA, :],
                                      in_=ga[:, :, EL:EL + 4])
                nc.vector.tensor_copy(out=ext[:, KA:KT, :],
                                      in_=gb[:, :, EL:EL + 4])
                nc.vector.tensor_add(ext[:, :, :], ext[:, :, :], er_ps[:, :, :])
                nc.vector.scalar_tensor_tensor(
                    out=ext[:, :, :], in0=ext[:, :, :], scalar=SLOPE,
                    in1=ext[:, :, :], op0=OP.mult, op1=OP.max)
                nc.scalar.activation(ext[:, :, :], ext[:, :, :], AF.Exp)
                ext16 = wp.tile([P, KT, 4], f16, tag="ext16")
                nc.vector.tensor_copy(out=ext16[:, :, :], in_=ext[:, :, :])

                rhs = wp.tile([P, KT, RHS], f16, tag="rhs")
                nc.vector.tensor_tensor(
                    out=rhs[:, 0:KA, 0:F].rearrange("p t (h d) -> p t h d", h=H),
                    in0=ga[:, :, 0:F].rearrange("p t (h d) -> p t h d", h=H),
                    in1=ext16[:, 0:KA, :, None].to_broadcast([P, KA, H, D]),
                    op=OP.mult)
                nc.vector.tensor_tensor(
                    out=rhs[:, KA:KT, 0:F].rearrange("p t (h d) -> p t h d", h=H),
                    in0=gb[:, :, 0:F].rearrange("p t (h d) -> p t h d", h=H),
                    in1=ext16[:, KA:KT, :, None].to_broadcast([P, KB, H, D]),
                    op=OP.mult)
                nc.vector.tensor_copy(out=rhs[:, :, F:F + 4], in_=ext16[:, :, :])

                psw = pp.tile([P, RHS], f32, tag="psw")
                for t in range(KT):
                    nc.tensor.matmul(out=psw[:, :], lhsT=oh[:, t, :],
                                     rhs=rhs[:, t, :],
                                     start=(t == 0), stop=(t == KT - 1))

                if dbg:
                    _tap("d_GA", ga[:, :, :], [P, KA, ROW])
                    _tap("d_GB", gb[:, :, :], [P, KB, ROW])
                    _tap("d_oh", oh[:, :, :], [P, KT, P])
                    _tap("d_ohT", ohT[:, :, :], [P, KT, P])
                    erps_s = wp.tile([P, KT, 4], f32, tag="erps_s")
                    nc.vector.tensor_copy(out=erps_s[:], in_=er_ps[:, :, :])
                    _tap("d_erps", erps_s[:, :, :], [P, KT, 4])
                    _tap("d_ext16", ext16[:, :, :], [P, KT, 4])
                    _tap("d_rhs", rhs[:, :, :], [P, KT, RHS])
                    psw_s = wp.tile([P, RHS], f32, tag="psw_s")
                    nc.vector.tensor_copy(out=psw_s[:], in_=psw[:, :])
                    _tap("d_psw", psw_s[:], [P, RHS])

                dn = wp.tile([P, 4], f32, tag="dn")
                if l < 2:
                    nc.vector.tensor_scalar_max(dn[:], psw[:, F:F + 4], 1e-30)
                else:
                    nc.vector.tensor_scalar(dn[:], psw[:, F:F + 4],
                                            1e-30, float(H), OP.max, OP.mult)
                rec = wp.tile([P, 4], f32, tag="rec")
                nc.vector.reciprocal(rec[:], dn[:])

                of = wp.tile([P, F], f32, tag="of")
                nc.vector.tensor_tensor(
                    out=of[:].rearrange("p (h d) -> p h d", h=H),
                    in0=psw[:, 0:F].rearrange("p (h d) -> p h d", h=H),
                    in1=rec[:, :, None].to_broadcast([P, H, D]),
                    op=OP.mult)
                if l == 1:
                    rt = wp.tile([P, F0], f32, tag="rt")
                    nc.vector.tensor_copy(out=rt[:],
                                          in_=h_sb[:, w * F0:(w + 1) * F0])
                    nc.vector.tensor_add(of[:], of[:], rt[:])
                elif l == 2:
                    rt = wp.tile([P, F2], f32, tag="rt")
                    nc.vector.tensor_copy(out=rt[:],
                                          in_=res2_sb[:, w * F2:(w + 1) * F2])
                    nc.vector.tensor_add(of[:], of[:], rt[:])

                if l < 2:
                    # ELU: out = (x - 1 - min(x,0)) + exp(min(x,0))
                    t0 = wp.tile([P, F0], f32, tag="t0")
                    nc.vector.tensor_scalar_min(t0[:], of[:], 0.0)
                    oh_out = wp.tile([P, F0], f32, tag="oh_out")
                    nc.vector.scalar_tensor_tensor(
                        out=oh_out[:], in0=of[:], scalar=-1.0, in1=t0[:],
                        op0=OP.add, op1=OP.subtract)
                    nc.scalar.activation(t0[:], t0[:], AF.Exp)
                    nc.vector.tensor_add(h_sb[:, w * F0:(w + 1) * F0],
                                         oh_out[:], t0[:])
                else:
                    msum = wp.tile([P, C], f32, tag="msum")
                    nc.vector.tensor_reduce(
                        msum[:],
                        of[:].rearrange("p (h c) -> p c h", h=H),
                        axis=mybir.AxisListType.X, op=OP.add)
                    nc.sync.dma_start(out_d[w * P:w * P + nw, :], msum[0:nw, :])

            def allgather(l):
                nc.gpsimd.collective_compute(
                    "AllGather", OP.bypass,
                    replica_groups=[list(range(NCORES))],
                    ins=[tin_rows[l][:, :].opt()],
                    outs=[tab[l][:, :].opt()])

            for w in range(W):
                dense_win(0, w, load_x=True)
            if dbg:
                _tap("d_tin0", tin_rows[0][0:P, :], [P, ROW0])
                _tap("d_er0", er_all[0][:, 0:8], [P, 8])
            allgather(0)
            if dbg:
                _tap("d_tabA", tab[0][0:P, :], [P, ROW0])
                _tap("d_tabB", tab[0][SPLIT:SPLIT + P, :], [P, ROW0])
            for l in range(3):
                for w in range(W):
                    edge_win(l, w, dbg=(dbg and l == 0 and w == 0))
                    if l < 2:
                        dense_win(l + 1, w)
                if l < 2:
                    allgather(l + 1)

    nc.compile()
    return nc


# ======================= entry point =======================

def kernel(**inputs) -> np.ndarray:
    from concourse.bass_utils import run_bass_kernel_spmd

    in_maps, meta_prog = _preprocess(inputs)
    key = (meta_prog["KA"], meta_prog["KB"])
    if key not in _CACHE:
        _CACHE[key] = _build(meta_prog)
    nc = _CACHE[key]
    res = run_bass_kernel_spmd(nc, in_maps, core_ids=list(range(NCORES)))
    return np.concatenate([r["out"] for r in res.results], axis=0)


# revision 14
# speedup vs baseline: 1.4476x; 1.4476x over previous
"""3-layer GAT on 8 Trainium2 NeuronCores (Bass/Tile), v2.

Sharding: nodes by contiguous range (6250/core); edges by dst range, sorted
by (dst, src). Per layer: dense phase computes [feat|el|er] = h @ [W|W.al|W.ar]
for local nodes into an fp16 row table -> AllGather the table -> edge phase
gathers table[src] rows (dma_gather on 4 SWDGE queues so descriptor generation
runs on all four Q7 core pairs), builds one-hot(dst) matrices on DVE in fp16,
broadcasts er via DMA-transposed one-hots + tiny matmuls, computes
exp(leaky_relu(el+er)) on ACT, and aggregates (weighted feature sum + softmax
denominator) in one fp16 matmul chain per 128-dst-node window into PSUM.
Epilogue normalizes, adds residual, applies ELU (or head-mean for the output
layer) and writes the next layer's input directly into SBUF (h never
round-trips DRAM). Dense phase of layer l+1 is interleaved with edge phase of
layer l per window.

All feature storage/movement is fp16 (table rows 768B / 512B vs f32 1280B),
halving gather+AllGather traffic; accumulation stays f32 in PSUM.

Softmax is computed without the segment-max subtraction: attention logits are
O(1) here so exp() cannot overflow, and the result is mathematically
identical.
"""
import sys

sys.path.insert(0, "/opt/trn_rl_repo")

import numpy as np

# ---- problem constants (nn_GAT_3951369912452) ----
N = 50000
E = 800000
IN = 256
HID = 64
H = 4
C = 40
SLOPE = 0.2
NCORES = 8
NLOC = N // NCORES          # 6250
P = 128
W = (NLOC + P - 1) // P     # 49 windows/core
SPLIT = 32768               # int16 gather index limit

F0 = H * HID                # 256 feat width, layers 0/1
F2 = H * C                  # 160 feat width, layer 2
ROW0 = 384                  # fp16 row elems, layers 0/1 (768B)
ROW2 = 256                  # fp16 row elems, layer 2 (512B)
EL0 = 256                   # el col offset in rows / psd, layers 0/1
EL2 = 160                   # layer 2
CD = (F0 + 8, F0 + 8, F2 + 8 + F2)   # dense matmul out cols per layer
ROWS = (ROW0, ROW0, ROW2)
ELS = (EL0, EL0, EL2)
FS = (F0, F0, F2)
RHSS = (F0 + 4, F0 + 4, F2 + 4)

_CACHE = {}


# ======================= host preprocessing =======================

def _fold_w(Wm, al, ar):
    Hh, D = al.shape
    Wal = np.stack([Wm[:, h * D:(h + 1) * D] @ al[h] for h in range(Hh)], axis=1)
    War = np.stack([Wm[:, h * D:(h + 1) * D] @ ar[h] for h in range(Hh)], axis=1)
    return Wal.astype(np.float32), War.astype(np.float32)


def _wrap16(block):
    """int16 idx list (cap,) -> dma_gather wrapped layout (128, cap//16)."""
    cap = block.shape[0]
    wb = block.reshape(cap // 16, 16).T
    return np.tile(wb, (8, 1)).astype(np.int16)


def _preprocess(inputs):
    x = np.asarray(inputs["x"], np.float32)
    src = np.asarray(inputs["src"], np.int64)
    dst = np.asarray(inputs["dst"], np.int64)

    Wcat = []
    for l, (Wm, al, ar) in enumerate(
        [(inputs["W0"], inputs["al0"], inputs["ar0"]),
         (inputs["W1"], inputs["al1"], inputs["ar1"]),
         (inputs["W2"], inputs["al2"], inputs["ar2"])]
    ):
        Wm = np.asarray(Wm, np.float32)
        Wal, War = _fold_w(Wm, np.asarray(al, np.float32), np.asarray(ar, np.float32))
        parts = [Wm, Wal, War]
        if l == 2:
            # residual projection, pre-scaled by the head-mean 1/H factor
            parts.append(np.asarray(inputs["Wres2"], np.float32) / H)
        Wcat.append(np.ascontiguousarray(
            np.concatenate(parts, axis=1)).astype(np.float16))

    order = np.argsort(dst, kind="stable")
    ds = dst[order]
    ss = src[order]

    # per (core, window) edge lists, split at SPLIT, sorted by src
    per_core = []
    KA = KB = 1
    for r in range(NCORES):
        lo = r * NLOC
        m = (ds >= lo) & (ds < lo + NLOC)
        ld = ds[m] - lo
        ls = ss[m]
        wins = []
        for w in range(W):
            wm = (ld >= w * P) & (ld < (w + 1) * P)
            dw = ld[wm] - w * P
            sw = ls[wm]
            so = np.argsort(sw, kind="stable")
            sw, dw = sw[so], dw[so]
            a = sw < SPLIT
            sa, da = sw[a], dw[a]
            sb, db = sw[~a] - SPLIT, dw[~a]
            wins.append((sa, da, sb, db))
            KA = max(KA, -(-max(len(sa), 1) // P))
            KB = max(KB, -(-max(len(sb), 1) // P))
        per_core.append(wins)
    KT = KA + KB

    in_maps = []
    for r in range(NCORES):
        idxA = np.full((W, KA * P), -1, np.int16)
        idxB = np.full((W, KB * P), -1, np.int16)
        drel = np.full((W, KT * P), -1.0, np.float32)
        meta = np.zeros((2 * W,), np.int32)
        for w, (sa, da, sb, db) in enumerate(per_core[r]):
            na, nb = len(sa), len(sb)
            idxA[w, :na] = sa.astype(np.int16)
            idxB[w, :nb] = sb.astype(np.int16)
            if na == 0:
                idxA[w, 0] = 0
            if nb == 0:
                idxB[w, 0] = 0
            meta[w] = max(na, 1)
            meta[W + w] = max(nb, 1)
            drel[w, :na] = da.astype(np.float32)
            drel[w, KA * P:KA * P + nb] = db.astype(np.float32)

        # drelF: per window, edge (lane p, tile t) value at flat col t*128+p,
        # replicated across all 128 partitions (for the transposed one-hot)
        drelF = np.broadcast_to(drel.reshape(1, W * KT * P),
                                (P, W * KT * P)).astype(np.float16)
        in_maps.append({
            "x": np.ascontiguousarray(x[r * NLOC:(r + 1) * NLOC]).astype(np.float16),
            "Wcat0": Wcat[0], "Wcat1": Wcat[1], "Wcat2": Wcat[2],
            "idxA": np.hstack([_wrap16(idxA[w]) for w in range(W)]),
            "idxB": np.hstack([_wrap16(idxB[w]) for w in range(W)]),
            "drel": np.hstack([drel[w].reshape(KT, P).T for w in range(W)]
                              ).astype(np.float16),
            "drelF": np.ascontiguousarray(drelF),
            "meta": meta.reshape(1, 2 * W),
        })

    meta_prog = {"KA": KA, "KB": KB}
    return in_maps, meta_prog


# ======================= device program =======================

def _build(meta_prog, dbg=False):
    import concourse.bass as bass
    import concourse.bacc as bacc
    import concourse.mybir as mybir
    import concourse.tile as tile

    KA, KB = meta_prog["KA"], meta_prog["KB"]
    KT = KA + KB
    f32 = mybir.dt.float32
    f16 = mybir.dt.float16
    i16 = mybir.dt.int16
    i32 = mybir.dt.int32
    AF = mybir.ActivationFunctionType
    OP = mybir.AluOpType

    nc = bacc.Bacc("TRN2", target_bir_lowering=False, debug=False,
                   num_devices=NCORES, num_swdge_queues=4)

    # ---- I/O ----
    x_d = nc.dram_tensor("x", [NLOC, F0], f16, kind="ExternalInput")
    Wc_d = [nc.dram_tensor(f"Wcat{l}", [IN, CD[l]], f16, kind="ExternalInput")
            for l in range(3)]
    idxA_d = nc.dram_tensor("idxA", [P, W * KA * 8], i16, kind="ExternalInput")
    idxB_d = nc.dram_tensor("idxB", [P, W * KB * 8], i16, kind="ExternalInput")
    drel_d = nc.dram_tensor("drel", [P, W * KT], f16, kind="ExternalInput")
    drelF_d = nc.dram_tensor("drelF", [P, W * KT * P], f16, kind="ExternalInput")
    meta_d = nc.dram_tensor("meta", [1, 2 * W], i32, kind="ExternalInput")
    out_d = nc.dram_tensor("out", [NLOC, C], f32, kind="ExternalOutput")

    # ---- internal DRAM ----
    tinf = [nc.dram_tensor(f"tin{l}", [(NLOC + 2) * ROWS[l]], f16)
            for l in range(3)]
    tab = [nc.dram_tensor(f"tab{l}", [N, ROWS[l]], f16, addr_space="Shared")
           for l in range(3)]
    tin_rows = [tinf[l][0:NLOC * ROWS[l]].rearrange("(n c) -> n c", c=ROWS[l])
                for l in range(3)]

    with tile.TileContext(nc) as tc:
        with (
            tc.tile_pool(name="const", bufs=1) as cp,
            tc.tile_pool(name="work", bufs=2) as wp,
            tc.tile_pool(name="psum", bufs=2, space="PSUM") as pp,
        ):
            # ---- persistent tiles ----
            iota_f = cp.tile([P, P], f32)
            nc.gpsimd.iota(iota_f[:], pattern=[[1, P]], base=0,
                           channel_multiplier=0,
                           allow_small_or_imprecise_dtypes=True)
            iota16 = cp.tile([P, P], f16)
            nc.vector.tensor_copy(out=iota16[:], in_=iota_f[:])
            iotap_f = cp.tile([P, 1], f32)
            nc.gpsimd.iota(iotap_f[:], pattern=[[0, 1]], base=0,
                           channel_multiplier=1,
                           allow_small_or_imprecise_dtypes=True)
            iotap16 = cp.tile([P, 1], f16)
            nc.vector.tensor_copy(out=iotap16[:], in_=iotap_f[:])

            idxA_t = cp.tile([P, W * KA * 8], i16)
            nc.sync.dma_start(idxA_t[:], idxA_d[:, :])
            idxB_t = cp.tile([P, W * KB * 8], i16)
            nc.sync.dma_start(idxB_t[:], idxB_d[:, :])
            drel_t = cp.tile([P, W * KT], f16)
            nc.sync.dma_start(drel_t[:], drel_d[:, :])
            meta_t = cp.tile([1, 2 * W], i32)
            nc.sync.dma_start(meta_t[:], meta_d[:, :])

            Wc_t = []
            for l in range(3):
                chunks = []
                for k in range(2):
                    t = cp.tile([P, CD[l]], f16, tag=f"wc{l}{k}")
                    nc.sync.dma_start(t[:], Wc_d[l][k * P:(k + 1) * P, :])
                    chunks.append(t)
                Wc_t.append(chunks)

            # h buffer, SBUF-resident across all layers
            h_sb = cp.tile([P, W * F0], f16)
            # zero the last window once so its pad lanes (beyond the x/h DMA)
            # stay finite (er columns feed matmuls)
            nc.vector.memset(h_sb[:, (W - 1) * F0:W * F0].bitcast(f32), 0.0)

            er_all = [cp.tile([P, W * 4], f32, name=f"er_all{i}", tag=f"er{i}")
                      for i in range(2)]
            res2_sb = cp.tile([P, W * F2], f16)

            # gather destinations: zeroed once; stale lanes are masked by the
            # one-hot (0 rows) so they only need to stay finite
            GA = []
            GB = []
            for i in range(2):
                ga = cp.tile([P, KA * ROW0], f16, tag=f"GA{i}")
                nc.vector.memset(ga[:].bitcast(f32), 0.0)
                GA.append(ga)
                gb = cp.tile([P, KB * ROW0], f16, tag=f"GB{i}")
                nc.vector.memset(gb[:].bitcast(f32), 0.0)
                GB.append(gb)

            regs = [nc.gpsimd.alloc_register(f"gr{i}") for i in range(8)]

            def _tap(name, ap, shape):
                d = nc.dram_tensor(name, shape, ap.dtype, kind="ExternalOutput")
                nc.sync.dma_start(d[tuple(slice(0, s) for s in shape)], ap)

            def dense_win(l, w, load_x=False):
                nw = min(P, NLOC - w * P)
                if load_x:
                    nc.sync.dma_start(h_sb[0:nw, w * F0:(w + 1) * F0],
                                      x_d[w * P:w * P + nw, :])
                psd = pp.tile([P, CD[l]], f32, tag="psd")
                for k in range(2):
                    hT = wp.tile([P, P], f16, tag="hT")
                    nc.scalar.dma_start_transpose(
                        hT[:], h_sb[:, w * F0 + k * P:w * F0 + (k + 1) * P])
                    nc.tensor.matmul(out=psd[:], lhsT=hT[:], rhs=Wc_t[l][k][:],
                                     start=(k == 0), stop=(k == 1))
                EL = ELS[l]
                nc.vector.tensor_copy(out=er_all[l % 2][:, w * 4:(w + 1) * 4],
                                      in_=psd[:, EL + 4:EL + 8])
                do = wp.tile([P, ROW0], f16, tag="do")
                nc.vector.tensor_copy(out=do[:, 0:EL + 4], in_=psd[:, 0:EL + 4])
                nc.sync.dma_start(tin_rows[l][w * P:w * P + nw, :],
                                  do[0:nw, 0:ROWS[l]])
                if l == 2:
                    nc.vector.tensor_copy(
                        out=res2_sb[:, w * F2:(w + 1) * F2],
                        in_=psd[:, EL + 8:EL + 8 + F2])

            def edge_win(l, w, dbg=False):
                ROW, EL, F, RHS = ROWS[l], ELS[l], FS[l], RHSS[l]
                D = F // H
                par = w % 2
                nw = min(P, NLOC - w * P)
                ga = GA[par][:, 0:KA * ROW].rearrange("p (t c) -> p t c", c=ROW)
                gb = GB[par][:, 0:KB * ROW].rearrange("p (t c) -> p t c", c=ROW)
                rA = regs[(2 * w) % 8]
                rB = regs[(2 * w + 1) % 8]
                nc.gpsimd.reg_load(rA, meta_t[0:1, w:w + 1])
                nc.gpsimd.dma_gather(
                    out_ap=ga, in_ap=tab[l][0:SPLIT, :],
                    idxs_ap=idxA_t[:, w * KA * 8:(w + 1) * KA * 8],
                    num_idxs=KA * P, num_idxs_reg=rA, elem_size=ROW,
                    single_packet=False, queue_num=2 * par)
                nc.gpsimd.reg_load(rB, meta_t[0:1, W + w:W + w + 1])
                nc.gpsimd.dma_gather(
                    out_ap=gb, in_ap=tab[l][SPLIT:N, :],
                    idxs_ap=idxB_t[:, w * KB * 8:(w + 1) * KB * 8],
                    num_idxs=KB * P, num_idxs_reg=rB, elem_size=ROW,
                    single_packet=False, queue_num=2 * par + 1)

                oh = wp.tile([P, KT, P], f16, tag="oh")
                nc.vector.tensor_tensor(
                    out=oh[:, :, :],
                    in0=iota16[:, None, :].to_broadcast([P, KT, P]),
                    in1=drel_t[:, w * KT:(w + 1) * KT, None].to_broadcast(
                        [P, KT, P]),
                    op=OP.is_equal)
                # transposed one-hot built directly: partition = dst, free = edge
                drelB = wp.tile([P, KT * P], f16, tag="drelB")
                nc.sync.dma_start(drelB[:],
                                  drelF_d[:, w * KT * P:(w + 1) * KT * P])
                ohT = wp.tile([P, KT, P], f16, tag="ohT")
                nc.vector.tensor_tensor(
                    out=ohT[:, :, :],
                    in0=iotap16[:, :, None].to_broadcast([P, KT, P]),
                    in1=drelB[:].rearrange("d (t p) -> d t p", p=P),
                    op=OP.is_equal)

                er_w = wp.tile([P, 4], f16, tag="er_w")
                nc.vector.tensor_copy(out=er_w[:],
                                      in_=er_all[l % 2][:, w * 4:(w + 1) * 4])
                er_ps = pp.tile([P, KT, 4], f32, tag="er_ps")
                for t in range(KT):
                    nc.tensor.matmul(out=er_ps[:, t, :], lhsT=ohT[:, t, :],
                                     rhs=er_w[:], start=True, stop=True)

                ext = wp.tile([P, KT, 4], f32, tag="ext")
                nc.vector.tensor_copy(out=ext[:, 0:KA, :],
                                      in_=ga[:, :, EL:EL + 4])
                nc.vector.tensor_copy(out=ext[:, KA:KT, :],
                                      in_=gb[:, :, EL:EL + 4])
                nc.vector.tensor_add(ext[:, :, :], ext[:, :, :], er_ps[:, :, :])
                nc.vector.scalar_tensor_tensor(
                    out=ext[:, :, :], in0=ext[:, :, :], scalar=SLOPE,
                    in1=ext[:, :, :], op0=OP.mult, op1=OP.max)
                nc.scalar.activation(ext[:, :, :], ext[:, :, :], AF.Exp)
                ext16 = wp.tile([P, KT, 4], f16, tag="ext16")
                nc.vector.tensor_copy(out=ext16[:, :, :], in_=ext[:, :, :])

                rhs = wp.tile([P, KT, RHS], f16, tag="rhs")
                nc.vector.tensor_tensor(
                    out=rhs[:, 0:KA, 0:F].rearrange("p t (h d) -> p t h d", h=H),
                    in0=ga[:, :, 0:F].rearrange("p t (h d) -> p t h d", h=H),
                    in1=ext16[:, 0:KA, :, None].to_broadcast([P, KA, H, D]),
                    op=OP.mult)
                nc.vector.tensor_tensor(
                    out=rhs[:, KA:KT, 0:F].rearrange("p t (h d) -> p t h d", h=H),
                    in0=gb[:, :, 0:F].rearrange("p t (h d) -> p t h d", h=H),
                    in1=ext16[:, KA:KT, :, None].to_broadcast([P, KB, H, D]),
                    op=OP.mult)
                nc.vector.tensor_copy(out=rhs[:, :, F:F + 4], in_=ext16[:, :, :])

                psw = pp.tile([P, RHS], f32, tag="psw")
                for t in range(KT):
                    nc.tensor.matmul(out=psw[:, :], lhsT=oh[:, t, :],
                                     rhs=rhs[:, t, :],
                                     start=(t == 0), stop=(t == KT - 1))

                if dbg:
                    _tap("d_GA", ga[:, :, :], [P, KA, ROW])
                    _tap("d_GB", gb[:, :, :], [P, KB, ROW])
                    _tap("d_oh", oh[:, :, :], [P, KT, P])
                    _tap("d_ohT", ohT[:, :, :], [P, KT, P])
                    erps_s = wp.tile([P, KT, 4], f32, tag="erps_s")
                    nc.vector.tensor_copy(out=erps_s[:], in_=er_ps[:, :, :])
                    _tap("d_erps", erps_s[:, :, :], [P, KT, 4])
                    _tap("d_ext16", ext16[:, :, :], [P, KT, 4])
                    _tap("d_rhs", rhs[:, :, :], [P, KT, RHS])
                    psw_s = wp.tile([P, RHS], f32, tag="psw_s")
                    nc.vector.tensor_copy(out=psw_s[:], in_=psw[:, :])
                    _tap("d_psw", psw_s[:], [P, RHS])

                dn = wp.tile([P, 4], f32, tag="dn")
                if l < 2:
                    nc.vector.tensor_scalar_max(dn[:], psw[:, F:F + 4], 1e-30)
                else:
                    nc.vector.tensor_scalar(dn[:], psw[:, F:F + 4],
                                            1e-30, float(H), OP.max, OP.mult)
                rec = wp.tile([P, 4], f32, tag="rec")
                nc.vector.reciprocal(rec[:], dn[:])

                of = wp.tile([P, F], f32, tag="of")
                nc.vector.tensor_tensor(
                    out=of[:].rearrange("p (h d) -> p h d", h=H),
                    in0=psw[:, 0:F].rearrange("p (h d) -> p h d", h=H),
                    in1=rec[:, :, None].to_broadcast([P, H, D]),
                    op=OP.mult)
                if l == 1:
                    rt = wp.tile([P, F0], f32, tag="rt")
                    nc.vector.tensor_copy(out=rt[:],
                                          in_=h_sb[:, w * F0:(w + 1) * F0])
                    nc.vector.tensor_add(of[:], of[:], rt[:])
                elif l == 2:
                    rt = wp.tile([P, F2], f32, tag="rt")
                    nc.vector.tensor_copy(out=rt[:],
                                          in_=res2_sb[:, w * F2:(w + 1) * F2])
                    nc.vector.tensor_add(of[:], of[:], rt[:])

                if l < 2:
                    # ELU: out = (x - 1 - min(x,0)) + exp(min(x,0))
                    t0 = wp.tile([P, F0], f32, tag="t0")
                    nc.vector.tensor_scalar_min(t0[:], of[:], 0.0)
                    oh_out = wp.tile([P, F0], f32, tag="oh_out")
                    nc.vector.scalar_tensor_tensor(
                        out=oh_out[:], in0=of[:], scalar=-1.0, in1=t0[:],
                        op0=OP.add, op1=OP.subtract)
                    nc.scalar.activation(t0[:], t0[:], AF.Exp)
                    nc.vector.tensor_add(h_sb[:, w * F0:(w + 1) * F0],
                                         oh_out[:], t0[:])
                else:
                    msum = wp.tile([P, C], f32, tag="msum")
                    nc.vector.tensor_reduce(
                        msum[:],
                        of[:].rearrange("p (h c) -> p c h", h=H),
                        axis=mybir.AxisListType.X, op=OP.add)
                    nc.sync.dma_start(out_d[w * P:w * P + nw, :], msum[0:nw, :])

            def allgather(l):
                nc.gpsimd.collective_compute(
                    "AllGather", OP.bypass,
                    replica_groups=[list(range(NCORES))],
                    ins=[tin_rows[l][:, :].opt()],
                    outs=[tab[l][:, :].opt()])

            for w in range(W):
                dense_win(0, w, load_x=True)
            if dbg:
                _tap("d_tin0", tin_rows[0][0:P, :], [P, ROW0])
                _tap("d_er0", er_all[0][:, 0:8], [P, 8])
            allgather(0)
            if dbg:
                _tap("d_tabA", tab[0][0:P, :], [P, ROW0])
                _tap("d_tabB", tab[0][SPLIT:SPLIT + P, :], [P, ROW0])
            for l in range(3):
                for w in range(W):
                    edge_win(l, w, dbg=(dbg and l == 0 and w == 0))
                    if l < 2:
                        dense_win(l + 1, w)
                if l < 2:
                    allgather(l + 1)

    nc.compile()
    return nc


# ======================= entry point =======================

def kernel(**inputs) -> np.ndarray:
    from concourse.bass_utils import run_bass_kernel_spmd

    in_maps, meta_prog = _preprocess(inputs)
    key = (meta_prog["KA"], meta_prog["KB"])
    if key not in _CACHE:
        _CACHE[key] = _build(meta_prog)
    nc = _CACHE[key]
    res = run_bass_kernel_spmd(nc, in_maps, core_ids=list(range(NCORES)))
    return np.concatenate([r["out"] for r in res.results], axis=0)
